# revision 7
# baseline (speedup 1.0000x reference)
"""Causal attention (B=4, S=2048, D=1024, single head) on 8 TRN2 NeuronCores.

Sharding: data-parallel over batch x causal-balanced query split.
  core c -> batch b = c//2, role r = c%2.
  Sequence is split into 4 blocks of 512 rows. Role 0 takes query blocks
  {0, 3}, role 1 takes {1, 2}: both roles attend to exactly 5 (q,k)
  block pairs, so the causal work is balanced.

One SPMD NEFF runs on all 8 cores. Per-core differences (which query rows,
which keys are causally visible) are carried in the input data only:
  - xqt:  x^T columns of this core's 1024 query rows
  - qidx: global sequence index of each local query row (f32)
The kernel computes K/V projections for the full 2048-row prefix, Q for its
1024 rows, then block attention where query half A visits key blocks {0,1}
and half B visits {0,1,2,3}; a data-driven additive mask
(-1e6 where kpos > qidx) realizes causality/padding uniformly.

Compute is bf16 on the TensorEngine with f32 PSUM accumulation; softmax
skips the running-max (logits are ~N(0,1) after the 1/32 scale, masked
lanes sit at -31250 and underflow to exactly 0).
"""

import sys

if "/opt/trn_rl_repo" not in sys.path:
    sys.path.insert(0, "/opt/trn_rl_repo")

import ml_dtypes
import numpy as np

import bass_rust

import concourse.bass as bass
import concourse.mybir as mybir
from concourse.masks import make_identity
from concourse.tile import TileContext

B, S, D = 4, 2048, 1024
P = 128
NCORES = 8
DC = D // P           # 8 contraction chunks of 128
QROWS = S // 2        # 1024 query rows per core
QT = QROWS // P       # 8 query tiles of 128 rows
KBLK = 512            # key block size
NKB = S // KBLK       # 4 key blocks
SCALE = 1.0 / np.sqrt(np.float32(D))
MASK_NEG = -1.0e6

F32 = mybir.dt.float32
BF16 = mybir.dt.bfloat16


# ---------------------------------------------------------------------------
# This container's walrus build (setupSyncWait, CoreV2/V3GenImpl.cpp) rejects
# any instruction carrying more than one sem wait. Tile's wait-assignment
# freely emits several. Hoist all but one wait of each instruction onto NOPs
# inserted immediately before it on the same engine — the engine executes its
# stream in order, so waiting on a preceding same-engine NOP is equivalent.
def _split_multi_waits(nc):
    n_split = 0
    for fn in nc.m.functions:
        for bb in fn.blocks:
            insts = list(bb.instructions)
            out = []
            changed = False
            for inst in insts:
                si = inst.sync_info
                if si is not None and len(si.on_wait) > 1:
                    waits = list(si.on_wait)
                    for w in waits[:-1]:
                        nop = mybir.InstNoOp(
                            name=f"{inst.name}-wsplit{n_split}", ins=[], outs=[]
                        )
                        n_split += 1
                        nop.engine = inst.engine
                        nop.sync_info = bass_rust.SyncInfo(
                            on_wait=[w], on_update=[]
                        )
                        out.append(nop)
                    inst.sync_info = bass_rust.SyncInfo(
                        on_wait=[waits[-1]], on_update=list(si.on_update)
                    )
                    changed = True
                if si is not None and len(si.on_update) > 2:
                    raise RuntimeError(
                        f"{inst.name}: {len(si.on_update)} sync updates; "
                        "update-splitting not implemented"
                    )
                out.append(inst)
            if changed:
                bb.instructions = out
    return nc
# ---------------------------------------------------------------------------


def _build_nc():
    nc = bass.Bass()

    xt = nc.declare_dram_parameter("xt", [D, S], BF16, isOutput=False)
    xqt = nc.declare_dram_parameter("xqt", [D, QROWS], BF16, isOutput=False)
    wq = nc.declare_dram_parameter("wq", [D, D], BF16, isOutput=False)
    wk = nc.declare_dram_parameter("wk", [D, D], BF16, isOutput=False)
    wv = nc.declare_dram_parameter("wv", [D, D], BF16, isOutput=False)
    qidx = nc.declare_dram_parameter("qidx", [QROWS], F32, isOutput=False)
    out = nc.declare_dram_parameter("out", [QROWS, D], F32, isOutput=True)

    xt_r = xt.rearrange("(dc p) s -> p dc s", p=P)
    xqt_r = xqt.rearrange("(dc p) s -> p dc s", p=P)
    wq_r = wq.rearrange("(dc p) e -> p dc e", p=P)
    wk_r = wk.rearrange("(dc p) e -> p dc e", p=P)
    wv_r = wv.rearrange("(dc p) e -> p dc e", p=P)
    qidx_r = qidx.rearrange("(t p) -> p t", p=P)

    with TileContext(nc) as tc:
        # Long-lived tiles: projected tensors + constants.
        persist = tc.alloc_tile_pool(name="persist", bufs=1)
        qt_sb = persist.tile([P, DC, QROWS], BF16, tag="qt_sb")     # Q^T [e, q]
        kt_sb = persist.tile([P, DC, S], BF16, tag="kt_sb")         # K^T [e, s]
        v_sb = persist.tile([P, S // P, D], BF16, tag="v_sb")       # V   [s, e]
        kpos_f = persist.tile([P, S], F32, tag="kpos_f")
        qidx_sb = persist.tile([P, QT], F32, tag="qidx_sb")
        ident = persist.tile([P, P], BF16, tag="ident")

        kpos_i = persist.tile([P, S], mybir.dt.int32, tag="kpos_i")
        nc.gpsimd.iota(kpos_i[:], pattern=[[1, S]], base=0, channel_multiplier=0)
        nc.vector.tensor_copy(kpos_f[:], kpos_i[:])
        nc.sync.dma_start(qidx_sb[:], qidx_r)
        make_identity(nc, ident[:])

        # ---- Phase 1: projections (scoped so the x/W staging frees) ----
        with (
            tc.tile_pool(name="proj_in", bufs=1) as proj_in,
            tc.tile_pool(name="proj_w", bufs=2) as proj_w,
            tc.tile_pool(name="proj_ps", bufs=4, space="PSUM") as proj_ps,
        ):
            xt_sb = proj_in.tile([P, DC, S], BF16, tag="xt_sb")
            xqt_sb = proj_in.tile([P, DC, QROWS], BF16, tag="xqt_sb")
            for dc in range(DC):
                nc.sync.dma_start(xt_sb[:, dc, :], xt_r[:, dc, :])
            for dc in range(0, DC, 2):
                nc.sync.dma_start(xqt_sb[:, dc : dc + 2, :], xqt_r[:, dc : dc + 2, :])

            def proj_T(w_sb, x_sb, dst, s_len):
                # dst[e, s] = W^T @ x^T : lhsT = W[d, e] chunk, rhs = x^T[d, s]
                for et in range(DC):
                    for sc in range(s_len // KBLK):
                        ps = proj_ps.tile([P, KBLK], F32, tag="proj_ps")
                        for dc in range(DC):
                            nc.tensor.matmul(
                                ps[:],
                                w_sb[:, dc, et * P : (et + 1) * P],
                                x_sb[:, dc, sc * KBLK : (sc + 1) * KBLK],
                                start=(dc == 0),
                                stop=(dc == DC - 1),
                            )
                        nc.scalar.copy(
                            dst[:, et, sc * KBLK : (sc + 1) * KBLK], ps[:]
                        )

            wq_sb = proj_w.tile([P, DC, D], BF16, tag="w")
            for dc in range(0, DC, 2):
                nc.sync.dma_start(wq_sb[:, dc : dc + 2, :], wq_r[:, dc : dc + 2, :])
            proj_T(wq_sb, xqt_sb, qt_sb, QROWS)

            wk_sb = proj_w.tile([P, DC, D], BF16, tag="w")
            for dc in range(0, DC, 2):
                nc.sync.dma_start(wk_sb[:, dc : dc + 2, :], wk_r[:, dc : dc + 2, :])
            proj_T(wk_sb, xt_sb, kt_sb, S)

            wv_sb = proj_w.tile([P, DC, D], BF16, tag="w")
            for dc in range(0, DC, 2):
                nc.sync.dma_start(wv_sb[:, dc : dc + 2, :], wv_r[:, dc : dc + 2, :])
            # V[s, e] : lhsT = x^T[d, s] chunk, rhs = Wv[d, e]
            for st in range(S // P):
                for ec in range(D // KBLK):
                    ps = proj_ps.tile([P, KBLK], F32, tag="proj_ps")
                    for dc in range(DC):
                        nc.tensor.matmul(
                            ps[:],
                            xt_sb[:, dc, st * P : (st + 1) * P],
                            wv_sb[:, dc, ec * KBLK : (ec + 1) * KBLK],
                            start=(dc == 0),
                            stop=(dc == DC - 1),
                        )
                    nc.scalar.copy(v_sb[:, st, ec * KBLK : (ec + 1) * KBLK], ps[:])

        # ---- Phase 2: block attention ----
        with (
            tc.tile_pool(name="att", bufs=2) as att,
            tc.tile_pool(name="att_sm", bufs=3) as att_sm,
            tc.tile_pool(name="ps_sc", bufs=2, space="PSUM") as ps_sc,
            tc.tile_pool(name="ps_pt", bufs=2, space="PSUM") as ps_pt,
            tc.tile_pool(name="ps_ctx", bufs=2, space="PSUM") as ps_ctx,
        ):
            for qt in range(QT):
                nvis = 2 if qt < QT // 2 else 4
                nkc = nvis * (KBLK // P)
                p_sb = att.tile([P, S], BF16, tag="p_sb")
                pt_sb = att.tile([P, S // P, P], BF16, tag="pt_sb")
                sums = att_sm.tile([P, NKB], F32, tag="sums")
                qcol = qidx_sb[:, qt : qt + 1]

                for v in range(nvis):
                    ksl = slice(v * KBLK, (v + 1) * KBLK)
                    sc_ps = ps_sc.tile([P, KBLK], F32, tag="sc_ps")
                    for ec in range(DC):
                        nc.tensor.matmul(
                            sc_ps[:],
                            qt_sb[:, ec, qt * P : (qt + 1) * P],
                            kt_sb[:, ec, ksl],
                            start=(ec == 0),
                            stop=(ec == DC - 1),
                        )
                    bias = att_sm.tile([P, KBLK], F32, tag="bias")
                    nc.vector.tensor_scalar(
                        bias[:], kpos_f[:, ksl], qcol, MASK_NEG,
                        mybir.AluOpType.is_gt, mybir.AluOpType.mult,
                    )
                    sm = att_sm.tile([P, KBLK], F32, tag="sm")
                    nc.vector.tensor_add(sm[:], sc_ps[:], bias[:])
                    nc.scalar.activation(
                        p_sb[:, ksl], sm[:],
                        mybir.ActivationFunctionType.Exp,
                        scale=float(SCALE),
                        accum_out=sums[:, v : v + 1],
                    )

                for kc in range(nkc):
                    pt_ps = ps_pt.tile([P, P], BF16, tag="pt_ps")
                    nc.tensor.transpose(
                        pt_ps[:], p_sb[:, kc * P : (kc + 1) * P], ident[:]
                    )
                    nc.vector.tensor_copy(pt_sb[:, kc, :], pt_ps[:])

                tot = att_sm.tile([P, 1], F32, tag="tot")
                rinv = att_sm.tile([P, 1], F32, tag="rinv")
                nc.vector.reduce_sum(
                    tot[:], sums[:, :nvis], axis=mybir.AxisListType.X
                )
                nc.vector.reciprocal(rinv[:], tot[:])

                ctx_lo = ps_ctx.tile([P, KBLK], F32, tag="ctx_lo")
                ctx_hi = ps_ctx.tile([P, KBLK], F32, tag="ctx_hi")
                for kc in range(nkc):
                    nc.tensor.matmul(
                        ctx_lo[:], pt_sb[:, kc, :], v_sb[:, kc, 0:KBLK],
                        start=(kc == 0), stop=(kc == nkc - 1),
                    )
                    nc.tensor.matmul(
                        ctx_hi[:], pt_sb[:, kc, :], v_sb[:, kc, KBLK:D],
                        start=(kc == 0), stop=(kc == nkc - 1),
                    )

                out_sb = att.tile([P, D], F32, tag="out_sb")
                nc.vector.tensor_scalar_mul(out_sb[:, 0:KBLK], ctx_lo[:], rinv[:])
                nc.vector.tensor_scalar_mul(out_sb[:, KBLK:D], ctx_hi[:], rinv[:])
                nc.sync.dma_start(out[qt * P : (qt + 1) * P, :], out_sb[:])

        persist.release()

    return _split_multi_waits(nc)


_NC_CACHE = None


def _get_nc():
    global _NC_CACHE
    if _NC_CACHE is None:
        _NC_CACHE = _build_nc()
    return _NC_CACHE


def _qrows(role):
    if role == 0:
        return np.concatenate([np.arange(0, KBLK), np.arange(3 * KBLK, S)])
    return np.arange(KBLK, 3 * KBLK)


def _shard_inputs(x, Wq, Wk, Wv):
    bf = ml_dtypes.bfloat16
    w = {
        "wq": np.ascontiguousarray(Wq.astype(bf)),
        "wk": np.ascontiguousarray(Wk.astype(bf)),
        "wv": np.ascontiguousarray(Wv.astype(bf)),
    }
    in_maps = []
    for c in range(NCORES):
        b, r = c // 2, c % 2
        rows = _qrows(r)
        xbT = np.ascontiguousarray(x[b].T.astype(bf))          # [D, S]
        xqT = np.ascontiguousarray(xbT[:, rows])               # [D, QROWS]
        in_maps.append(
            {
                "xt": xbT,
                "xqt": xqT,
                "qidx": rows.astype(np.float32),
                **w,
            }
        )
    return in_maps


def _unshard(results, dtype):
    out = np.empty((B, S, D), dtype=dtype)
    for c in range(NCORES):
        b, r = c // 2, c % 2
        out[b, _qrows(r), :] = results[c]["out"]
    return out


def run(x, Wq, Wk, Wv, trace=False, tmpdir=None):
    from concourse.bass_utils import run_bass_kernel_spmd

    nc = _get_nc()
    in_maps = _shard_inputs(x, Wq, Wk, Wv)
    res = run_bass_kernel_spmd(
        nc, in_maps, core_ids=list(range(NCORES)), trace=trace, tmpdir=tmpdir
    )
    return _unshard(res.results, np.dtype(x.dtype)), res


def kernel(x, Wq, Wk, Wv):
    out, _ = run(np.asarray(x), np.asarray(Wq), np.asarray(Wk), np.asarray(Wv))
    return out


# revision 11
# speedup vs baseline: 1.0969x; 1.0969x over previous
"""Causal attention (B=4, S=2048, D=1024, single head) on 8 TRN2 NeuronCores.

Sharding: data-parallel over batch x causal-balanced query split.
  core c -> batch b = c//2, role r = c%2.
  Sequence is split into 4 blocks of 512 rows. Role 0 takes query blocks
  {0, 3}, role 1 takes {1, 2}: both roles attend to exactly 5 (q,k)
  block pairs, so the causal work is balanced.

One SPMD NEFF runs on all 8 cores. Per-core differences (which query rows,
which keys are causally visible) are carried in the input data only:
  - xqt:  x^T columns of this core's 1024 query rows
  - qidx: global sequence index of each local query row (f32)
The kernel computes K/V projections for the full 2048-row prefix, Q for its
1024 rows, then block attention where query half A visits key blocks {0,1}
and half B visits {0,1,2,3}; a data-driven additive mask
(-1e6 where kpos > qidx) realizes causality/padding uniformly.

Compute is bf16 on the TensorEngine with f32 PSUM accumulation; softmax
skips the running-max (logits are ~N(0,1) after the 1/32 scale, masked
lanes sit at -31250 and underflow to exactly 0).
"""

import sys

if "/opt/trn_rl_repo" not in sys.path:
    sys.path.insert(0, "/opt/trn_rl_repo")

import ml_dtypes
import numpy as np

import bass_rust

import concourse.bass as bass
import concourse.mybir as mybir
from concourse.masks import make_identity
from concourse.tile import TileContext

B, S, D = 4, 2048, 1024
P = 128
NCORES = 8
DC = D // P           # 8 contraction chunks of 128
QROWS = S // 2        # 1024 query rows per core
QT = QROWS // P       # 8 query tiles of 128 rows
KBLK = 512            # key block size
NKB = S // KBLK       # 4 key blocks
SCALE = 1.0 / np.sqrt(np.float32(D))
MASK_NEG = -1.0e6

F32 = mybir.dt.float32
BF16 = mybir.dt.bfloat16


# ---------------------------------------------------------------------------
# This container's walrus build (setupSyncWait, CoreV2/V3GenImpl.cpp) rejects
# any instruction carrying more than one sem wait. Tile's wait-assignment
# freely emits several. Hoist all but one wait of each instruction onto NOPs
# inserted immediately before it on the same engine — the engine executes its
# stream in order, so waiting on a preceding same-engine NOP is equivalent.
def _split_multi_waits(nc):
    n_split = 0
    for fn in nc.m.functions:
        for bb in fn.blocks:
            insts = list(bb.instructions)
            out = []
            changed = False
            for inst in insts:
                si = inst.sync_info
                if si is not None and len(si.on_wait) > 1:
                    waits = list(si.on_wait)
                    for w in waits[:-1]:
                        nop = mybir.InstNoOp(
                            name=f"{inst.name}-wsplit{n_split}", ins=[], outs=[]
                        )
                        n_split += 1
                        nop.engine = inst.engine
                        nop.sync_info = bass_rust.SyncInfo(
                            on_wait=[w], on_update=[]
                        )
                        out.append(nop)
                    inst.sync_info = bass_rust.SyncInfo(
                        on_wait=[waits[-1]], on_update=list(si.on_update)
                    )
                    changed = True
                if si is not None and len(si.on_update) > 2:
                    raise RuntimeError(
                        f"{inst.name}: {len(si.on_update)} sync updates; "
                        "update-splitting not implemented"
                    )
                out.append(inst)
            if changed:
                bb.instructions = out
    return nc
# ---------------------------------------------------------------------------


def _build_nc():
    nc = bass.Bass()

    xt = nc.declare_dram_parameter("xt", [D, S], BF16, isOutput=False)
    xqt = nc.declare_dram_parameter("xqt", [D, QROWS], BF16, isOutput=False)
    wq = nc.declare_dram_parameter("wq", [D, D], BF16, isOutput=False)
    wk = nc.declare_dram_parameter("wk", [D, D], BF16, isOutput=False)
    wv = nc.declare_dram_parameter("wv", [D, D], BF16, isOutput=False)
    qidx = nc.declare_dram_parameter("qidx", [QROWS], F32, isOutput=False)
    out = nc.declare_dram_parameter("out", [QROWS, D], F32, isOutput=True)

    xt_r = xt.rearrange("(dc p) s -> p dc s", p=P)
    xqt_r = xqt.rearrange("(dc p) s -> p dc s", p=P)
    wq_r = wq.rearrange("(dc p) e -> p dc e", p=P)
    wk_r = wk.rearrange("(dc p) e -> p dc e", p=P)
    wv_r = wv.rearrange("(dc p) e -> p dc e", p=P)
    qidx_r = qidx.rearrange("(t p) -> p t", p=P)

    with TileContext(nc) as tc:
        # Long-lived tiles: projected tensors + constants.
        persist = tc.alloc_tile_pool(name="persist", bufs=1)
        qt_sb = persist.tile([P, DC, QROWS], BF16, tag="qt_sb")     # Q^T [e, q]
        kt_sb = persist.tile([P, DC, S], BF16, tag="kt_sb")         # K^T [e, s]
        v_sb = persist.tile([P, S // P, D], BF16, tag="v_sb")       # V   [s, e]
        kpos_f = persist.tile([P, S], F32, tag="kpos_f")
        qidx_sb = persist.tile([P, QT], F32, tag="qidx_sb")
        ident = persist.tile([P, P], BF16, tag="ident")

        kpos_i = persist.tile([P, S], mybir.dt.int32, tag="kpos_i")
        nc.gpsimd.iota(kpos_i[:], pattern=[[1, S]], base=0, channel_multiplier=0)
        nc.vector.tensor_copy(kpos_f[:], kpos_i[:])
        nc.sync.dma_start(qidx_sb[:], qidx_r)
        make_identity(nc, ident[:])

        # ---- Phase 1: projections (scoped so the x/W staging frees) ----
        with (
            tc.tile_pool(name="proj_in", bufs=1) as proj_in,
            tc.tile_pool(name="proj_w", bufs=2) as proj_w,
            tc.tile_pool(name="proj_ps", bufs=4, space="PSUM") as proj_ps,
        ):
            xt_sb = proj_in.tile([P, DC, S], BF16, tag="xt_sb")
            xqt_sb = proj_in.tile([P, DC, QROWS], BF16, tag="xqt_sb")

            def proj_T(w_sb, x_sb, dst, s_len):
                # dst[e, s] = W^T @ x^T : lhsT = W[d, e] chunk, rhs = x^T[d, s]
                for et in range(DC):
                    for sc in range(s_len // KBLK):
                        ps = proj_ps.tile([P, KBLK], F32, tag="proj_ps")
                        for dc in range(DC):
                            nc.tensor.matmul(
                                ps[:],
                                w_sb[:, dc, et * P : (et + 1) * P],
                                x_sb[:, dc, sc * KBLK : (sc + 1) * KBLK],
                                start=(dc == 0),
                                stop=(dc == DC - 1),
                            )
                        nc.scalar.copy(
                            dst[:, et, sc * KBLK : (sc + 1) * KBLK], ps[:]
                        )

            # DMA order = first-use order: Q projection inputs first so the
            # TensorEngine starts as early as possible.
            wq_sb = proj_w.tile([P, DC, D], BF16, tag="w")
            for dc in range(0, DC, 2):
                nc.sync.dma_start(wq_sb[:, dc : dc + 2, :], wq_r[:, dc : dc + 2, :])
            for dc in range(0, DC, 2):
                nc.sync.dma_start(xqt_sb[:, dc : dc + 2, :], xqt_r[:, dc : dc + 2, :])
            wk_sb = proj_w.tile([P, DC, D], BF16, tag="w")
            for dc in range(0, DC, 2):
                nc.sync.dma_start(wk_sb[:, dc : dc + 2, :], wk_r[:, dc : dc + 2, :])
            for dc in range(DC):
                nc.sync.dma_start(xt_sb[:, dc, :], xt_r[:, dc, :])
            wv_sb = proj_w.tile([P, DC, D], BF16, tag="w")
            for dc in range(0, DC, 2):
                nc.sync.dma_start(wv_sb[:, dc : dc + 2, :], wv_r[:, dc : dc + 2, :])

            proj_T(wq_sb, xqt_sb, qt_sb, QROWS)
            proj_T(wk_sb, xt_sb, kt_sb, S)
            # V[s, e] : lhsT = x^T[d, s] chunk, rhs = Wv[d, e]
            for st in range(S // P):
                for ec in range(D // KBLK):
                    ps = proj_ps.tile([P, KBLK], F32, tag="proj_ps")
                    for dc in range(DC):
                        nc.tensor.matmul(
                            ps[:],
                            xt_sb[:, dc, st * P : (st + 1) * P],
                            wv_sb[:, dc, ec * KBLK : (ec + 1) * KBLK],
                            start=(dc == 0),
                            stop=(dc == DC - 1),
                        )
                    nc.scalar.copy(v_sb[:, st, ec * KBLK : (ec + 1) * KBLK], ps[:])

        # ---- Phase 2: block attention ----
        with (
            tc.tile_pool(name="att", bufs=2) as att,
            tc.tile_pool(name="att_sm", bufs=3) as att_sm,
            tc.tile_pool(name="ps_sc", bufs=2, space="PSUM") as ps_sc,
            tc.tile_pool(name="ps_pt", bufs=2, space="PSUM") as ps_pt,
            tc.tile_pool(name="ps_ctx", bufs=2, space="PSUM") as ps_ctx,
        ):
            for qt in range(QT):
                # 256-row slot s = qt//2 visits s+1 key blocks: both roles'
                # tile sets ({0,3,4,7} / {1,2,5,6} of 256-row tiles) have
                # visit-needs {1,2,3,4} — uniform program, zero padding.
                nvis = qt // 2 + 1
                nkc = nvis * (KBLK // P)
                p_sb = att.tile([P, S], BF16, tag="p_sb")
                pt_sb = att.tile([P, S // P, P], BF16, tag="pt_sb")
                sums = att_sm.tile([P, NKB], F32, tag="sums")
                qcol = qidx_sb[:, qt : qt + 1]

                for v in range(nvis):
                    ksl = slice(v * KBLK, (v + 1) * KBLK)
                    sc_ps = ps_sc.tile([P, KBLK], F32, tag="sc_ps")
                    for ec in range(DC):
                        nc.tensor.matmul(
                            sc_ps[:],
                            qt_sb[:, ec, qt * P : (qt + 1) * P],
                            kt_sb[:, ec, ksl],
                            start=(ec == 0),
                            stop=(ec == DC - 1),
                        )
                    bias = att_sm.tile([P, KBLK], F32, tag="bias")
                    nc.vector.tensor_scalar(
                        bias[:], kpos_f[:, ksl], qcol, MASK_NEG,
                        mybir.AluOpType.is_gt, mybir.AluOpType.mult,
                    )
                    sm = att_sm.tile([P, KBLK], F32, tag="sm")
                    nc.vector.tensor_add(sm[:], sc_ps[:], bias[:])
                    nc.scalar.activation(
                        p_sb[:, ksl], sm[:],
                        mybir.ActivationFunctionType.Exp,
                        scale=float(SCALE),
                        accum_out=sums[:, v : v + 1],
                    )

                for kc in range(nkc):
                    pt_ps = ps_pt.tile([P, P], BF16, tag="pt_ps")
                    nc.tensor.transpose(
                        pt_ps[:], p_sb[:, kc * P : (kc + 1) * P], ident[:]
                    )
                    nc.vector.tensor_copy(pt_sb[:, kc, :], pt_ps[:])

                tot = att_sm.tile([P, 1], F32, tag="tot")
                rinv = att_sm.tile([P, 1], F32, tag="rinv")
                nc.vector.reduce_sum(
                    tot[:], sums[:, :nvis], axis=mybir.AxisListType.X
                )
                nc.vector.reciprocal(rinv[:], tot[:])

                ctx_lo = ps_ctx.tile([P, KBLK], F32, tag="ctx_lo")
                ctx_hi = ps_ctx.tile([P, KBLK], F32, tag="ctx_hi")
                for kc in range(nkc):
                    nc.tensor.matmul(
                        ctx_lo[:], pt_sb[:, kc, :], v_sb[:, kc, 0:KBLK],
                        start=(kc == 0), stop=(kc == nkc - 1),
                    )
                    nc.tensor.matmul(
                        ctx_hi[:], pt_sb[:, kc, :], v_sb[:, kc, KBLK:D],
                        start=(kc == 0), stop=(kc == nkc - 1),
                    )

                out_sb = att.tile([P, D], F32, tag="out_sb")
                nc.vector.tensor_scalar_mul(out_sb[:, 0:KBLK], ctx_lo[:], rinv[:])
                nc.vector.tensor_scalar_mul(out_sb[:, KBLK:D], ctx_hi[:], rinv[:])
                nc.sync.dma_start(out[qt * P : (qt + 1) * P, :], out_sb[:])

        persist.release()

    return _split_multi_waits(nc)


_NC_CACHE = None


def _get_nc():
    global _NC_CACHE
    if _NC_CACHE is None:
        _NC_CACHE = _build_nc()
    return _NC_CACHE


_TILE256 = {0: (0, 3, 4, 7), 1: (1, 2, 5, 6)}


def _qrows(role):
    # 256-row tiles ordered by ascending visit-need (1,2,3,4 key blocks).
    return np.concatenate(
        [np.arange(t * 256, (t + 1) * 256) for t in _TILE256[role]]
    )


def _shard_inputs(x, Wq, Wk, Wv):
    bf = ml_dtypes.bfloat16
    w = {
        "wq": np.ascontiguousarray(Wq.astype(bf)),
        "wk": np.ascontiguousarray(Wk.astype(bf)),
        "wv": np.ascontiguousarray(Wv.astype(bf)),
    }
    in_maps = []
    for c in range(NCORES):
        b, r = c // 2, c % 2
        rows = _qrows(r)
        xbT = np.ascontiguousarray(x[b].T.astype(bf))          # [D, S]
        xqT = np.ascontiguousarray(xbT[:, rows])               # [D, QROWS]
        in_maps.append(
            {
                "xt": xbT,
                "xqt": xqT,
                "qidx": rows.astype(np.float32),
                **w,
            }
        )
    return in_maps


def _unshard(results, dtype):
    out = np.empty((B, S, D), dtype=dtype)
    for c in range(NCORES):
        b, r = c // 2, c % 2
        out[b, _qrows(r), :] = results[c]["out"]
    return out


def run(x, Wq, Wk, Wv, trace=False, tmpdir=None):
    from concourse.bass_utils import run_bass_kernel_spmd

    nc = _get_nc()
    in_maps = _shard_inputs(x, Wq, Wk, Wv)
    res = run_bass_kernel_spmd(
        nc, in_maps, core_ids=list(range(NCORES)), trace=trace, tmpdir=tmpdir
    )
    return _unshard(res.results, np.dtype(x.dtype)), res


def kernel(x, Wq, Wk, Wv):
    out, _ = run(np.asarray(x), np.asarray(Wq), np.asarray(Wk), np.asarray(Wv))
    return out
